# revision 10
# baseline (speedup 1.0000x reference)
"""GQA sliding-window+ALiBi (banded causal attention) on 8 TRN2 cores — v3.

Sharding: 8 cores = 2 batches x 4 kv-head groups (unchanged).
Per core: Q proj for 4 query heads, K/V proj for 1 kv head, banded
attention (window 1024), partial row-parallel Wo. Host sums partials.

v3 changes vs v2 (TimelineSim: PE busy was 212.6us of 233.1us total):
- softmax denominator no longer computed with per-kj PE matmuls
  (was 55k cycles = 23us of PE). Instead the masked exp tiles are
  accumulated on DVE (bf16, 2x mode) into pacc[128,512] per
  (group, head-pair); one matmul per (g,pair) with lhsT=ones[128,128]
  colsums pacc AND broadcasts den to all 128 partitions.
- that broadcast also kills the old bc matmul + bcs copy: reciprocal
  reads the [128,512] den psum directly and the tail muls consume it.
- attention operands (qT/kT/v/pt/masks) switched f32r -> bf16: same PE
  cost (1 cycle/row), half the DVE cost on mask muls / accumulate adds
  (2x mode needs all-2-byte operands), half the SBUF.
- phase-1 DMAs issued in consumption order on one queue (wq in 1024-col
  pieces interleaved with hst/wk/wv) so the first matmul starts ~2us
  earlier and mid-chunk stalls shrink; final output DMA split 512-wide.
"""
import math
from collections import deque
from contextlib import ExitStack

import numpy as np
import ml_dtypes

import concourse.tile as tile
from concourse import bacc, mybir
from concourse.bass_utils import run_bass_kernel_spmd
from concourse.masks import make_identity

dt = mybir.dt
bf16 = ml_dtypes.bfloat16

B, S, H = 2, 2048, 2048
NUM_HEADS, KV_HEADS, D = 16, 4, 128
WINDOW = 1024
GH = 4            # query heads per kv head (per core)
GD = GH * D       # 512: per-core slice of the hidden dim
SCALE = 1.0 / math.sqrt(D)
NG = 8            # 256-wide query groups

_nc_cache = None

# packed binary-mask tile layout: doubled full-width masks for o=0
# [diag|none] @0 and o=-7 [none|edge] @512, then doubled half-width
# masks [diag|diag] @1024 and [edge|edge] @1280 for the o=+1 / o=-8
# tiles (those only compute their valid 128-col half per head)
MASK_OFF = {0: 0, -7: 512}
HALF_MASK = {1: 1024, -8: 1280}


def _build_nc():
    nc = bacc.Bacc()
    hsd = nc.declare_dram_parameter("hsd", [4, 128, 8192], dt.bfloat16, isOutput=False)
    wqd = nc.declare_dram_parameter("wqd", [128, 8192], dt.bfloat16, isOutput=False)
    wkd = nc.declare_dram_parameter("wkd", [128, 2048], dt.bfloat16, isOutput=False)
    wvd = nc.declare_dram_parameter("wvd", [128, 2048], dt.bfloat16, isOutput=False)
    wod = nc.declare_dram_parameter("wod", [128, 8192], dt.bfloat16, isOutput=False)
    mskd = nc.declare_dram_parameter("mskd", [128, 1536], dt.bfloat16, isOutput=False)
    outd = nc.declare_dram_parameter("outd", [16, 128, 2048], dt.bfloat16, isOutput=True)

    with tile.TileContext(nc) as tc, ExitStack() as ctx:
        consts = ctx.enter_context(tc.tile_pool(name="consts", bufs=1))
        wpool = ctx.enter_context(tc.tile_pool(name="wpool", bufs=1))
        big = ctx.enter_context(tc.tile_pool(name="big", bufs=1))
        hstp = ctx.enter_context(tc.tile_pool(name="hstp", bufs=2))
        vtp = ctx.enter_context(tc.tile_pool(name="vtp", bufs=2))
        ptp = ctx.enter_context(tc.tile_pool(name="ptp", bufs=18))
        paccp = ctx.enter_context(tc.tile_pool(name="paccp", bufs=4))
        smalls = ctx.enter_context(tc.tile_pool(name="smalls", bufs=3))
        outp = ctx.enter_context(tc.tile_pool(name="outp", bufs=4))
        psum = ctx.enter_context(tc.tile_pool(name="psum", bufs=1, space="PSUM"))

        # constants
        ident32 = consts.tile([128, 128], dt.float32)
        make_identity(nc, ident32)
        ident = consts.tile([128, 128], dt.bfloat16)
        nc.vector.tensor_copy(ident, ident32)
        ones_bf = consts.tile([128, 128], dt.bfloat16)
        nc.vector.memset(ones_bf, 1.0)
        masks = consts.tile([128, 1536], dt.bfloat16)

        # weights
        wq = wpool.tile([128, 8192], dt.bfloat16, tag="wq")
        wk = wpool.tile([128, 2048], dt.bfloat16, tag="wk")
        wv = wpool.tile([128, 2048], dt.bfloat16, tag="wv")
        wo = wpool.tile([128, 8192], dt.bfloat16, tag="wo")

        # persistent activations
        # qT free layout (g, h, 256q): the attention rhs for a head PAIR at
        # query block g is then one contiguous 512-wide slice (a rearranged
        # strided view defeats subtile dependency tracking -> HW race)
        qT = big.tile([128, 4 * 2048], dt.bfloat16, tag="qT")
        kT = big.tile([128, 2048], dt.bfloat16, tag="kT")       # (d part, s)
        v = big.tile([128, 2048], dt.bfloat16, tag="v")         # (k part, (blk, d))
        ohT = big.tile([128, 4 * 2048], dt.bfloat16, tag="ohT")  # (d part, (h, s))

        # ---- Phase 1: projections, per 512-wide s-chunk ----
        for ch in range(4):
            hst = hstp.tile([128, 8192], dt.bfloat16, tag="hst")
            if ch == 0:
                # consumption-ordered startup: matmul t needs wq[:, t*512:*],
                # hst[:, t*512:*], wk/wv[:, t*128:*]. All on the SP queue: the
                # ACT queue opens with the 1.3us exp-table load, which would
                # delay every weight DMA behind it.
                nc.sync.dma_start(out=wq[:, 0:512], in_=wqd[:, 0:512])
                nc.sync.dma_start(out=hst[:, 0:512], in_=hsd[0][:, 0:512])
                nc.sync.dma_start(out=wk[:, 0:128], in_=wkd[:, 0:128])
                nc.sync.dma_start(out=wv[:, 0:128], in_=wvd[:, 0:128])
                nc.sync.dma_start(out=hst[:, 512:2048], in_=hsd[0][:, 512:2048])
                nc.sync.dma_start(out=wq[:, 512:1536], in_=wqd[:, 512:1536])
                nc.sync.dma_start(out=wk[:, 128:512], in_=wkd[:, 128:512])
                nc.sync.dma_start(out=wv[:, 128:512], in_=wvd[:, 128:512])
                nc.sync.dma_start(out=wq[:, 1536:2560], in_=wqd[:, 1536:2560])
                nc.sync.dma_start(out=hst[:, 2048:4096], in_=hsd[0][:, 2048:4096])
                nc.sync.dma_start(out=wq[:, 2560:3584], in_=wqd[:, 2560:3584])
                nc.sync.dma_start(out=wk[:, 512:2048], in_=wkd[:, 512:2048])
                nc.sync.dma_start(out=wv[:, 512:2048], in_=wvd[:, 512:2048])
                nc.sync.dma_start(out=hst[:, 4096:8192], in_=hsd[0][:, 4096:8192])
                for c0 in range(3584, 8192, 1024):
                    c1 = min(c0 + 1024, 8192)
                    nc.sync.dma_start(out=wq[:, c0:c1], in_=wqd[:, c0:c1])
            else:
                nc.sync.dma_start(out=hst[:, 0:4096], in_=hsd[ch][:, 0:4096])
                nc.sync.dma_start(out=hst[:, 4096:8192], in_=hsd[ch][:, 4096:8192])
            if ch == 1:
                # masks/wo are phase-2 inputs: keep the scheduler from
                # hoisting their DMAs ahead of phase-1's startup pieces
                with tc.high_priority(offset=-50000):
                    nc.scalar.dma_start(out=masks, in_=mskd[:, :])
            if ch >= 2:
                for i in (0, 1) if ch == 2 else (2, 3):
                    with tc.high_priority(offset=-50000):
                        nc.scalar.dma_start(out=wo[:, i * 2048:(i + 1) * 2048],
                                            in_=wod[:, i * 2048:(i + 1) * 2048])
            q_ps = [psum.tile([128, 512], dt.float32, tag="A", bufs=4, name=f"qps{ch}_{h}")
                    for h in range(GH)]
            k_ps = psum.tile([128, 512], dt.float32, tag="B", bufs=2, name=f"kps{ch}")
            v_ps = psum.tile([128, 512], dt.float32, tag="B", bufs=2, name=f"vps{ch}")
            for t in range(16):
                hs_t = hst[:, t * 512:(t + 1) * 512]
                st0, sp0 = (t == 0), (t == 15)
                for h in range(GH):
                    nc.tensor.matmul(q_ps[h],
                                     lhsT=wq[:, t * 512 + h * 128:t * 512 + (h + 1) * 128],
                                     rhs=hs_t, start=st0, stop=sp0)
                nc.tensor.matmul(k_ps, lhsT=wk[:, t * 128:(t + 1) * 128],
                                 rhs=hs_t, start=st0, stop=sp0)
                nc.tensor.matmul(v_ps, lhsT=wv[:, t * 128:(t + 1) * 128],
                                 rhs=hs_t, start=st0, stop=sp0)
            for h in range(GH):
                for gg in range(2):
                    g_abs = 2 * ch + gg
                    dst = qT[:, g_abs * 1024 + h * 256:g_abs * 1024 + (h + 1) * 256]
                    srcp = q_ps[h][:, gg * 256:(gg + 1) * 256]
                    if h % 2 == 0:
                        nc.vector.tensor_copy(dst, srcp)
                    else:
                        nc.scalar.copy(dst, srcp)
            nc.vector.tensor_copy(kT[:, ch * 512:(ch + 1) * 512], k_ps)
            vt = vtp.tile([128, 512], dt.bfloat16, tag="vt")
            nc.vector.tensor_copy(vt, v_ps)
            for j in range(4):
                tp = psum.tile([128, 512], dt.bfloat16, tag="C", bufs=2, name=f"tp{ch}_{j}")
                nc.tensor.transpose(tp[:, 0:128], vt[:, j * 128:(j + 1) * 128], ident)
                nc.scalar.copy(v[:, (4 * ch + j) * 128:(4 * ch + j + 1) * 128],
                               tp[:, 0:128])

        # ---- Phase 2/3: banded attention with Wo interleaved ----

        osb_cur = {}

        def emit_wo_half(st, ep):
            if ep == 0:
                osb_cur[st] = outp.tile([128, 2048], dt.bfloat16, tag="osb",
                                        name=f"osb{st}")
            osb = osb_cur[st]
            wops = [psum.tile([128, 512], dt.float32, tag="A", bufs=4,
                              name=f"wops{st}_{ep}_{i}") for i in range(2)]
            for ct in range(4):
                for i in range(2):
                    e = 2 * ep + i
                    nc.tensor.matmul(
                        wops[i],
                        lhsT=ohT[:, ct * 2048 + st * 128:ct * 2048 + (st + 1) * 128],
                        rhs=wo[:, ct * 2048 + e * 512:ct * 2048 + (e + 1) * 512],
                        start=(ct == 0), stop=(ct == 3))
            for i in range(2):
                e = 2 * ep + i
                if i == 0:
                    nc.scalar.copy(osb[:, e * 512:(e + 1) * 512], wops[i])
                else:
                    nc.vector.tensor_copy(osb[:, e * 512:(e + 1) * 512], wops[i])
            if st == 15:
                for e in (2 * ep, 2 * ep + 1):
                    nc.sync.dma_start(out=outd[st][:, e * 512:(e + 1) * 512],
                                      in_=osb[:, e * 512:(e + 1) * 512])
            else:
                # output DMAs have ~2 groups of slack (osb reuse at st+4);
                # keep them out of the way of input DMAs
                with tc.high_priority(offset=-5000):
                    nc.sync.dma_start(out=outd[st][:, ep * 1024:(ep + 1) * 1024],
                                      in_=osb[:, ep * 1024:(ep + 1) * 1024])

        state = {}

        def emit_tail_mul(g, pair):
            # ohT = av * rcs; rcs is 1/den already broadcast to all 128
            # partitions by the colsum matmul (lhsT = ones[128,128])
            av = state[(g, "av", pair)]
            rcs = state[(g, "rcs", pair)]
            for hh in range(2):
                h = pair * 2 + hh
                nc.vector.tensor_mul(
                    ohT[:, h * 2048 + g * 256:h * 2048 + (g + 1) * 256],
                    av[:, hh * 256:(hh + 1) * 256], rcs[:, hh * 256:(hh + 1) * 256])

        wo_q = deque()
        pend = deque()

        def emit_avden(item):
            g, kj, pair, pt, half, qoff, first, last, last_full = item
            if first:
                state[(g, "av", pair)] = psum.tile(
                    [128, 512], dt.float32, tag="B", bufs=2, name=f"av{g}_{pair}")
            av = state[(g, "av", pair)]
            if half:
                # accumulate into the valid 128-col half of each head segment
                av2 = av.rearrange("p (a b) -> p a b", a=2)
                pt2 = pt[:, 0:256].rearrange("p (a b) -> p a b", a=2)
                nc.tensor.matmul(av2[:, :, qoff:qoff + 128],
                                 lhsT=v[:, kj * 128:(kj + 1) * 128],
                                 rhs=pt2, start=False, stop=last,
                                 skip_group_check=True)
            else:
                nc.tensor.matmul(av, lhsT=v[:, kj * 128:(kj + 1) * 128],
                                 rhs=pt, start=first, stop=last_full,
                                 skip_group_check=True)
            if last:
                # colsum + broadcast of the DVE-accumulated exp sums: one
                # matmul, den replicated on every partition
                den = psum.tile([128, 512], dt.float32, tag="C", bufs=2,
                                name=f"den{g}_{pair}")
                nc.tensor.matmul(den, lhsT=ones_bf,
                                 rhs=state[(g, "pacc", pair)],
                                 start=True, stop=True)
                rcs = smalls.tile([128, 512], dt.float32r, tag="rcs",
                                  name=f"rcs{g}_{pair}")
                # recip + tails gate the next group's av psum slots: emit
                # them now, priority-hoisted past queued DVE bulk work
                with tc.high_priority():
                    with nc.allow_low_precision(reason="f32r is full fp32 bits"):
                        nc.vector.reciprocal(rcs, den)
                    state[(g, "rcs", pair)] = rcs
                    emit_tail_mul(g, pair)
                if pair == 1:
                    wo_q.extend([
                        (lambda g=g: emit_wo_half(2 * g, 0)),
                        (lambda g=g: emit_wo_half(2 * g, 1)),
                        (lambda g=g: emit_wo_half(2 * g + 1, 0)),
                        (lambda g=g: emit_wo_half(2 * g + 1, 1)),
                    ])

        def pops_ready():
            if len(pend) <= 11:
                return False
            if pend[0][6] and len(pend) <= 13:
                return False  # extra slack before a new group's first avden
            return True

        for g in range(NG):
            kjs = list(range(max(0, 2 * g - 8), 2 * g + 2))
            # valid-half-only tiles (o=+1 diag, o=-8 window edge) go last so
            # their av matmuls accumulate subranges after the full-width
            # chain has started
            fulls = [kj for kj in kjs if kj - 2 * g not in HALF_MASK]
            halves = sorted((kj for kj in kjs if kj - 2 * g in HALF_MASK),
                            key=lambda kj: kj - 2 * g)
            ordered = fulls + halves
            for i, kj in enumerate(ordered):
                o = kj - 2 * g
                half = o in HALF_MASK
                qoff = 128 if o == 1 else 0
                w = 256 if half else 512
                for pair in range(2):
                    base = g * 1024 + pair * 512
                    if half:
                        rhs = qT[:, base:base + 512].rearrange(
                            "p (a b) -> p a b", a=2)[:, :, qoff:qoff + 128]
                    else:
                        rhs = qT[:, base:base + 512]
                    sps = psum.tile([128, 512], dt.float32, tag="A", bufs=4,
                                    name=f"sps{g}_{kj}_{pair}")
                    nc.tensor.matmul(
                        sps[:, 0:w],
                        lhsT=kT[:, kj * 128:(kj + 1) * 128],
                        rhs=rhs, start=True, stop=True)
                    pt = ptp.tile([128, 512], dt.bfloat16, tag="pt",
                                  name=f"pt{g}_{kj}_{pair}")
                    nc.scalar.activation(pt[:, 0:w], sps[:, 0:w],
                                         mybir.ActivationFunctionType.Exp,
                                         scale=SCALE)
                    # binary window/causal mask applied post-exp:
                    # exp(s)*0 == exp(s + NEG), and keeps the DVE hop off
                    # the scores->exp chain that frees the sps psum slot
                    mo = HALF_MASK.get(o) if half else MASK_OFF.get(o)
                    if mo is not None:
                        nc.vector.tensor_mul(pt[:, 0:w], pt[:, 0:w],
                                             masks[:, mo:mo + w])
                    # denominator accumulation (bf16): pair0 chain on DVE
                    # (2x mode), pair1 chain on the otherwise-idle GpSimd
                    eng = nc.vector if pair == 0 else nc.gpsimd
                    if kj == ordered[0]:
                        pacc = paccp.tile([128, 512], dt.bfloat16, tag="pacc",
                                          name=f"pacc{g}_{pair}")
                        state[(g, "pacc", pair)] = pacc
                        eng.tensor_copy(pacc, pt)
                    else:
                        pacc = state[(g, "pacc", pair)]
                        with nc.allow_low_precision(reason="exp sums ~1e3, bf16 ok"):
                            if half:
                                pacc2 = pacc.rearrange("p (a b) -> p a b", a=2)
                                pt2 = pt[:, 0:256].rearrange("p (a b) -> p a b", a=2)
                                eng.tensor_add(
                                    pacc2[:, :, qoff:qoff + 128],
                                    pacc2[:, :, qoff:qoff + 128], pt2)
                            else:
                                eng.tensor_add(pacc, pacc, pt)
                    pend.append((g, kj, pair, pt, half, qoff,
                                 kj == ordered[0], kj == ordered[-1],
                                 kj == fulls[-1]))
                if wo_q:
                    # wo reads ohT written by the tail muls (data deps
                    # serialize them); one half per kj keeps PE fed
                    wo_q.popleft()()
                while pops_ready():
                    emit_avden(pend.popleft())
        while pend:
            emit_avden(pend.popleft())
        while wo_q:
            wo_q.popleft()()

    nc.compile()
    return nc


def _build_masks():
    kk = np.arange(128)[:, None]
    qq = np.arange(128)[None, :]
    diag = np.where(kk <= qq, 1.0, 0.0).astype(np.float32)
    edge = np.where(kk >= qq, 1.0, 0.0).astype(np.float32)
    none = np.ones((128, 128), np.float32)
    m0 = np.hstack([diag, none])   # o = 0 (full width)
    m3 = np.hstack([none, edge])   # o = -7 (full width)
    return np.hstack([m0, m0, m3, m3, diag, diag, edge, edge]).astype(bf16)


def kernel(hidden_states, Wq, Wk, Wv, Wo):
    global _nc_cache
    if _nc_cache is None:
        _nc_cache = _build_nc()
    nc = _nc_cache

    masks = _build_masks()
    hsd = []
    for b in range(B):
        ht = hidden_states[b].T.astype(bf16)                     # [H, S]
        t4 = ht.reshape(16, 128, 4, 512).transpose(2, 1, 0, 3)   # [ch, p, t, n]
        hsd.append(np.ascontiguousarray(t4.reshape(4, 128, 8192)))
    in_maps = []
    for b in range(B):
        for gi in range(KV_HEADS):
            wq = Wq[:, gi * GD:(gi + 1) * GD].astype(bf16)       # [2048, 512]
            wk = Wk[:, gi * D:(gi + 1) * D].astype(bf16)         # [2048, 128]
            wv = Wv[:, gi * D:(gi + 1) * D].astype(bf16)
            wo = Wo[gi * GD:(gi + 1) * GD, :].astype(bf16)       # [512, 2048]
            in_maps.append({
                "hsd": hsd[b],
                "wqd": np.ascontiguousarray(
                    wq.reshape(16, 128, 512).transpose(1, 0, 2).reshape(128, 8192)),
                "wkd": np.ascontiguousarray(
                    wk.reshape(16, 128, 128).transpose(1, 0, 2).reshape(128, 2048)),
                "wvd": np.ascontiguousarray(
                    wv.reshape(16, 128, 128).transpose(1, 0, 2).reshape(128, 2048)),
                "wod": np.ascontiguousarray(
                    wo.reshape(4, 128, 2048).transpose(1, 0, 2).reshape(128, 8192)),
                "mskd": masks,
            })
    res = run_bass_kernel_spmd(nc, in_maps, list(range(8)))
    out = np.zeros((B, S, H), np.float32)
    for b in range(B):
        acc = None
        for gi in range(KV_HEADS):
            o = np.asarray(res.results[b * KV_HEADS + gi]["outd"]).astype(np.float32)
            acc = o if acc is None else acc + o
        out[b] = acc.reshape(S, H)                               # [16,128,2048] -> [S,H]
    return out


# revision 12
# speedup vs baseline: 1.0287x; 1.0287x over previous
"""GQA sliding-window+ALiBi (banded causal attention) on 8 TRN2 cores — v3.

Sharding: 8 cores = 2 batches x 4 kv-head groups (unchanged).
Per core: Q proj for 4 query heads, K/V proj for 1 kv head, banded
attention (window 1024), partial row-parallel Wo. Host sums partials.

v3 changes vs v2 (TimelineSim: PE busy was 212.6us of 233.1us total):
- softmax denominator no longer computed with per-kj PE matmuls
  (was 55k cycles = 23us of PE). Instead the masked exp tiles are
  accumulated on DVE (bf16, 2x mode) into pacc[128,512] per
  (group, head-pair); one matmul per (g,pair) with lhsT=ones[128,128]
  colsums pacc AND broadcasts den to all 128 partitions.
- that broadcast also kills the old bc matmul + bcs copy: reciprocal
  reads the [128,512] den psum directly and the tail muls consume it.
- attention operands (qT/kT/v/pt/masks) switched f32r -> bf16: same PE
  cost (1 cycle/row), half the DVE cost on mask muls / accumulate adds
  (2x mode needs all-2-byte operands), half the SBUF.
- phase-1 DMAs issued in consumption order on one queue (wq in 1024-col
  pieces interleaved with hst/wk/wv) so the first matmul starts ~2us
  earlier and mid-chunk stalls shrink; final output DMA split 512-wide.
"""
import math
from collections import deque
from contextlib import ExitStack

import numpy as np
import ml_dtypes

import concourse.tile as tile
from concourse import bacc, mybir
from concourse.bass_utils import run_bass_kernel_spmd
from concourse.masks import make_identity

dt = mybir.dt
bf16 = ml_dtypes.bfloat16

B, S, H = 2, 2048, 2048
NUM_HEADS, KV_HEADS, D = 16, 4, 128
WINDOW = 1024
GH = 4            # query heads per kv head (per core)
GD = GH * D       # 512: per-core slice of the hidden dim
SCALE = 1.0 / math.sqrt(D)
NG = 8            # 256-wide query groups

_nc_cache = None

# packed binary-mask tile layout: doubled full-width masks for o=0
# [diag|none] @0 and o=-7 [none|edge] @512, then doubled half-width
# masks [diag|diag] @1024 and [edge|edge] @1280 for the o=+1 / o=-8
# tiles (those only compute their valid 128-col half per head)
MASK_OFF = {0: 0, -7: 512}
HALF_MASK = {1: 1024, -8: 1280}


def _build_nc():
    nc = bacc.Bacc()
    hsd = nc.declare_dram_parameter("hsd", [4, 128, 8192], dt.bfloat16, isOutput=False)
    wqd = nc.declare_dram_parameter("wqd", [128, 8192], dt.bfloat16, isOutput=False)
    wkd = nc.declare_dram_parameter("wkd", [128, 2048], dt.bfloat16, isOutput=False)
    wvd = nc.declare_dram_parameter("wvd", [128, 2048], dt.bfloat16, isOutput=False)
    wod = nc.declare_dram_parameter("wod", [128, 8192], dt.bfloat16, isOutput=False)
    mskd = nc.declare_dram_parameter("mskd", [128, 1536], dt.bfloat16, isOutput=False)
    outd = nc.declare_dram_parameter("outd", [16, 128, 2048], dt.bfloat16, isOutput=True)

    with tile.TileContext(nc) as tc, ExitStack() as ctx:
        consts = ctx.enter_context(tc.tile_pool(name="consts", bufs=1))
        wpool = ctx.enter_context(tc.tile_pool(name="wpool", bufs=1))
        big = ctx.enter_context(tc.tile_pool(name="big", bufs=1))
        hstp = ctx.enter_context(tc.tile_pool(name="hstp", bufs=2))
        vtp = ctx.enter_context(tc.tile_pool(name="vtp", bufs=2))
        ptp = ctx.enter_context(tc.tile_pool(name="ptp", bufs=18))
        paccp = ctx.enter_context(tc.tile_pool(name="paccp", bufs=4))
        smalls = ctx.enter_context(tc.tile_pool(name="smalls", bufs=3))
        outp = ctx.enter_context(tc.tile_pool(name="outp", bufs=4))
        psum = ctx.enter_context(tc.tile_pool(name="psum", bufs=1, space="PSUM"))

        # constants
        ident32 = consts.tile([128, 128], dt.float32)
        make_identity(nc, ident32)
        ident = consts.tile([128, 128], dt.bfloat16)
        nc.vector.tensor_copy(ident, ident32)
        ones_bf = consts.tile([128, 128], dt.bfloat16)
        nc.vector.memset(ones_bf, 1.0)
        masks = consts.tile([128, 1536], dt.bfloat16)

        # weights
        wq = wpool.tile([128, 8192], dt.bfloat16, tag="wq")
        wk = wpool.tile([128, 2048], dt.bfloat16, tag="wk")
        wv = wpool.tile([128, 2048], dt.bfloat16, tag="wv")
        wo = wpool.tile([128, 8192], dt.bfloat16, tag="wo")

        # persistent activations
        # qT free layout (g, h, 256q): the attention rhs for a head PAIR at
        # query block g is then one contiguous 512-wide slice (a rearranged
        # strided view defeats subtile dependency tracking -> HW race)
        qT = big.tile([128, 4 * 2048], dt.bfloat16, tag="qT")
        kT = big.tile([128, 2048], dt.bfloat16, tag="kT")       # (d part, s)
        v = big.tile([128, 2048], dt.bfloat16, tag="v")         # (k part, (blk, d))
        ohT = big.tile([128, 4 * 2048], dt.bfloat16, tag="ohT")  # (d part, (h, s))

        # ---- Phase 1: projections, per 512-wide s-chunk ----
        for ch in range(4):
            hst = hstp.tile([128, 8192], dt.bfloat16, tag="hst")
            if ch == 0:
                # consumption-ordered startup: matmul t needs wq[:, t*512:*],
                # hst[:, t*512:*], wk/wv[:, t*128:*]. All on the SP queue: the
                # ACT queue opens with the 1.3us exp-table load, which would
                # delay every weight DMA behind it.
                nc.sync.dma_start(out=wq[:, 0:512], in_=wqd[:, 0:512])
                nc.sync.dma_start(out=hst[:, 0:512], in_=hsd[0][:, 0:512])
                nc.sync.dma_start(out=wk[:, 0:128], in_=wkd[:, 0:128])
                nc.sync.dma_start(out=wv[:, 0:128], in_=wvd[:, 0:128])
                nc.sync.dma_start(out=hst[:, 512:2048], in_=hsd[0][:, 512:2048])
                nc.sync.dma_start(out=wq[:, 512:1536], in_=wqd[:, 512:1536])
                nc.sync.dma_start(out=wk[:, 128:512], in_=wkd[:, 128:512])
                nc.sync.dma_start(out=wv[:, 128:512], in_=wvd[:, 128:512])
                nc.sync.dma_start(out=wq[:, 1536:2560], in_=wqd[:, 1536:2560])
                nc.sync.dma_start(out=hst[:, 2048:4096], in_=hsd[0][:, 2048:4096])
                nc.sync.dma_start(out=wq[:, 2560:3584], in_=wqd[:, 2560:3584])
                nc.sync.dma_start(out=wk[:, 512:2048], in_=wkd[:, 512:2048])
                nc.sync.dma_start(out=wv[:, 512:2048], in_=wvd[:, 512:2048])
                nc.sync.dma_start(out=hst[:, 4096:8192], in_=hsd[0][:, 4096:8192])
                for c0 in range(3584, 8192, 1024):
                    c1 = min(c0 + 1024, 8192)
                    nc.sync.dma_start(out=wq[:, c0:c1], in_=wqd[:, c0:c1])
            else:
                nc.sync.dma_start(out=hst[:, 0:4096], in_=hsd[ch][:, 0:4096])
                nc.sync.dma_start(out=hst[:, 4096:8192], in_=hsd[ch][:, 4096:8192])
            if ch == 1:
                # masks/wo are phase-2 inputs: hold their DMAs back (the
                # scheduler otherwise hoists them ahead of phase-1's
                # startup pieces, starving the first chunk's matmuls)
                with tc.tile_wait_until(0.026):
                    nc.scalar.dma_start(out=masks, in_=mskd[:, :])
            if ch >= 2:
                for i in (0, 1) if ch == 2 else (2, 3):
                    with tc.tile_wait_until(0.030 + 0.012 * i):
                        nc.scalar.dma_start(out=wo[:, i * 2048:(i + 1) * 2048],
                                            in_=wod[:, i * 2048:(i + 1) * 2048])
            q_ps = [psum.tile([128, 512], dt.float32, tag="A", bufs=4, name=f"qps{ch}_{h}")
                    for h in range(GH)]
            k_ps = psum.tile([128, 512], dt.float32, tag="B", bufs=2, name=f"kps{ch}")
            v_ps = psum.tile([128, 512], dt.float32, tag="B", bufs=2, name=f"vps{ch}")
            for t in range(16):
                hs_t = hst[:, t * 512:(t + 1) * 512]
                st0, sp0 = (t == 0), (t == 15)
                for h in range(GH):
                    nc.tensor.matmul(q_ps[h],
                                     lhsT=wq[:, t * 512 + h * 128:t * 512 + (h + 1) * 128],
                                     rhs=hs_t, start=st0, stop=sp0)
                nc.tensor.matmul(k_ps, lhsT=wk[:, t * 128:(t + 1) * 128],
                                 rhs=hs_t, start=st0, stop=sp0)
                nc.tensor.matmul(v_ps, lhsT=wv[:, t * 128:(t + 1) * 128],
                                 rhs=hs_t, start=st0, stop=sp0)
            for h in range(GH):
                for gg in range(2):
                    g_abs = 2 * ch + gg
                    dst = qT[:, g_abs * 1024 + h * 256:g_abs * 1024 + (h + 1) * 256]
                    srcp = q_ps[h][:, gg * 256:(gg + 1) * 256]
                    if h % 2 == 0:
                        nc.vector.tensor_copy(dst, srcp)
                    else:
                        nc.scalar.copy(dst, srcp)
            nc.vector.tensor_copy(kT[:, ch * 512:(ch + 1) * 512], k_ps)
            vt = vtp.tile([128, 512], dt.bfloat16, tag="vt")
            nc.vector.tensor_copy(vt, v_ps)
            for j in range(4):
                tp = psum.tile([128, 512], dt.bfloat16, tag="C", bufs=2, name=f"tp{ch}_{j}")
                nc.tensor.transpose(tp[:, 0:128], vt[:, j * 128:(j + 1) * 128], ident)
                nc.scalar.copy(v[:, (4 * ch + j) * 128:(4 * ch + j + 1) * 128],
                               tp[:, 0:128])

        # ---- Phase 2/3: banded attention with Wo interleaved ----

        osb_cur = {}

        def emit_wo_half(st, ep):
            if ep == 0:
                osb_cur[st] = outp.tile([128, 2048], dt.bfloat16, tag="osb",
                                        name=f"osb{st}")
            osb = osb_cur[st]
            if st == 15:
                # final tile: fully pipeline matmul->copy->dma per 512 so
                # the kernel tail is one 512-col chain, not the whole ep
                for i in range(2):
                    e = 2 * ep + i
                    wop = psum.tile([128, 512], dt.float32, tag="A", bufs=4,
                                    name=f"wops{st}_{ep}_{i}")
                    for ct in range(4):
                        nc.tensor.matmul(
                            wop,
                            lhsT=ohT[:, ct * 2048 + st * 128:ct * 2048 + (st + 1) * 128],
                            rhs=wo[:, ct * 2048 + e * 512:ct * 2048 + (e + 1) * 512],
                            start=(ct == 0), stop=(ct == 3))
                    if i == 0:
                        nc.scalar.copy(osb[:, e * 512:(e + 1) * 512], wop)
                    else:
                        nc.vector.tensor_copy(osb[:, e * 512:(e + 1) * 512], wop)
                    nc.sync.dma_start(out=outd[st][:, e * 512:(e + 1) * 512],
                                      in_=osb[:, e * 512:(e + 1) * 512])
                return
            wops = [psum.tile([128, 512], dt.float32, tag="A", bufs=4,
                              name=f"wops{st}_{ep}_{i}") for i in range(2)]
            for ct in range(4):
                for i in range(2):
                    e = 2 * ep + i
                    nc.tensor.matmul(
                        wops[i],
                        lhsT=ohT[:, ct * 2048 + st * 128:ct * 2048 + (st + 1) * 128],
                        rhs=wo[:, ct * 2048 + e * 512:ct * 2048 + (e + 1) * 512],
                        start=(ct == 0), stop=(ct == 3))
            for i in range(2):
                e = 2 * ep + i
                if i == 0:
                    nc.scalar.copy(osb[:, e * 512:(e + 1) * 512], wops[i])
                else:
                    nc.vector.tensor_copy(osb[:, e * 512:(e + 1) * 512], wops[i])
            nc.sync.dma_start(out=outd[st][:, ep * 1024:(ep + 1) * 1024],
                              in_=osb[:, ep * 1024:(ep + 1) * 1024])

        state = {}

        def emit_tail_mul(g, pair):
            # ohT = av * rcs; rcs is 1/den already broadcast to all 128
            # partitions by the colsum matmul (lhsT = ones[128,128])
            av = state[(g, "av", pair)]
            rcs = state[(g, "rcs", pair)]
            for hh in range(2):
                h = pair * 2 + hh
                nc.vector.tensor_mul(
                    ohT[:, h * 2048 + g * 256:h * 2048 + (g + 1) * 256],
                    av[:, hh * 256:(hh + 1) * 256], rcs[:, hh * 256:(hh + 1) * 256])

        wo_q = deque()
        pend = deque()

        def emit_avden(item):
            g, kj, pair, pt, half, qoff, first, last, last_full = item
            if first:
                state[(g, "av", pair)] = psum.tile(
                    [128, 512], dt.float32, tag="B", bufs=2, name=f"av{g}_{pair}")
            av = state[(g, "av", pair)]
            if half:
                # accumulate into the valid 128-col half of each head segment
                av2 = av.rearrange("p (a b) -> p a b", a=2)
                pt2 = pt[:, 0:256].rearrange("p (a b) -> p a b", a=2)
                nc.tensor.matmul(av2[:, :, qoff:qoff + 128],
                                 lhsT=v[:, kj * 128:(kj + 1) * 128],
                                 rhs=pt2, start=False, stop=last,
                                 skip_group_check=True)
            else:
                nc.tensor.matmul(av, lhsT=v[:, kj * 128:(kj + 1) * 128],
                                 rhs=pt, start=first, stop=last_full,
                                 skip_group_check=True)
            if last:
                # colsum + broadcast of the DVE-accumulated exp sums: one
                # matmul, den replicated on every partition
                den = psum.tile([128, 512], dt.float32, tag="C", bufs=2,
                                name=f"den{g}_{pair}")
                nc.tensor.matmul(den, lhsT=ones_bf,
                                 rhs=state[(g, "pacc", pair)],
                                 start=True, stop=True)
                rcs = smalls.tile([128, 512], dt.float32r, tag="rcs",
                                  name=f"rcs{g}_{pair}")
                # recip + tails gate the next group's av psum slots: emit
                # them now, priority-hoisted past queued DVE bulk work
                with tc.high_priority():
                    with nc.allow_low_precision(reason="f32r is full fp32 bits"):
                        nc.vector.reciprocal(rcs, den)
                    state[(g, "rcs", pair)] = rcs
                    emit_tail_mul(g, pair)
                if pair == 1:
                    wo_q.extend([
                        (lambda g=g: emit_wo_half(2 * g, 0)),
                        (lambda g=g: emit_wo_half(2 * g, 1)),
                        (lambda g=g: emit_wo_half(2 * g + 1, 0)),
                        (lambda g=g: emit_wo_half(2 * g + 1, 1)),
                    ])

        def pops_ready():
            if len(pend) <= 11:
                return False
            if pend[0][6] and len(pend) <= 13:
                return False  # extra slack before a new group's first avden
            return True

        for g in range(NG):
            kjs = list(range(max(0, 2 * g - 8), 2 * g + 2))
            # valid-half-only tiles (o=+1 diag, o=-8 window edge) go last so
            # their av matmuls accumulate subranges after the full-width
            # chain has started
            fulls = [kj for kj in kjs if kj - 2 * g not in HALF_MASK]
            halves = sorted((kj for kj in kjs if kj - 2 * g in HALF_MASK),
                            key=lambda kj: kj - 2 * g)
            ordered = fulls + halves
            for i, kj in enumerate(ordered):
                o = kj - 2 * g
                half = o in HALF_MASK
                qoff = 128 if o == 1 else 0
                w = 256 if half else 512
                for pair in range(2):
                    base = g * 1024 + pair * 512
                    if half:
                        rhs = qT[:, base:base + 512].rearrange(
                            "p (a b) -> p a b", a=2)[:, :, qoff:qoff + 128]
                    else:
                        rhs = qT[:, base:base + 512]
                    sps = psum.tile([128, 512], dt.float32, tag="A", bufs=4,
                                    name=f"sps{g}_{kj}_{pair}")
                    nc.tensor.matmul(
                        sps[:, 0:w],
                        lhsT=kT[:, kj * 128:(kj + 1) * 128],
                        rhs=rhs, start=True, stop=True)
                    pt = ptp.tile([128, 512], dt.bfloat16, tag="pt",
                                  name=f"pt{g}_{kj}_{pair}")
                    nc.scalar.activation(pt[:, 0:w], sps[:, 0:w],
                                         mybir.ActivationFunctionType.Exp,
                                         scale=SCALE)
                    # binary window/causal mask applied post-exp:
                    # exp(s)*0 == exp(s + NEG), and keeps the DVE hop off
                    # the scores->exp chain that frees the sps psum slot
                    mo = HALF_MASK.get(o) if half else MASK_OFF.get(o)
                    if mo is not None:
                        nc.vector.tensor_mul(pt[:, 0:w], pt[:, 0:w],
                                             masks[:, mo:mo + w])
                    # denominator accumulation (bf16): pair0 chain on DVE
                    # (2x mode), pair1 chain on the otherwise-idle GpSimd
                    eng = nc.vector if pair == 0 else nc.gpsimd
                    if kj == ordered[0]:
                        pacc = paccp.tile([128, 512], dt.bfloat16, tag="pacc",
                                          name=f"pacc{g}_{pair}")
                        state[(g, "pacc", pair)] = pacc
                        eng.tensor_copy(pacc, pt)
                    else:
                        pacc = state[(g, "pacc", pair)]
                        with nc.allow_low_precision(reason="exp sums ~1e3, bf16 ok"):
                            if half:
                                pacc2 = pacc.rearrange("p (a b) -> p a b", a=2)
                                pt2 = pt[:, 0:256].rearrange("p (a b) -> p a b", a=2)
                                eng.tensor_add(
                                    pacc2[:, :, qoff:qoff + 128],
                                    pacc2[:, :, qoff:qoff + 128], pt2)
                            else:
                                eng.tensor_add(pacc, pacc, pt)
                    pend.append((g, kj, pair, pt, half, qoff,
                                 kj == ordered[0], kj == ordered[-1],
                                 kj == fulls[-1]))
                if wo_q:
                    # wo reads ohT written by the tail muls (data deps
                    # serialize them); one half per kj keeps PE fed
                    wo_q.popleft()()
                while pops_ready():
                    emit_avden(pend.popleft())
        while pend:
            emit_avden(pend.popleft())
        while wo_q:
            wo_q.popleft()()

    nc.compile()
    return nc


def _build_masks():
    kk = np.arange(128)[:, None]
    qq = np.arange(128)[None, :]
    diag = np.where(kk <= qq, 1.0, 0.0).astype(np.float32)
    edge = np.where(kk >= qq, 1.0, 0.0).astype(np.float32)
    none = np.ones((128, 128), np.float32)
    m0 = np.hstack([diag, none])   # o = 0 (full width)
    m3 = np.hstack([none, edge])   # o = -7 (full width)
    return np.hstack([m0, m0, m3, m3, diag, diag, edge, edge]).astype(bf16)


def kernel(hidden_states, Wq, Wk, Wv, Wo):
    global _nc_cache
    if _nc_cache is None:
        _nc_cache = _build_nc()
    nc = _nc_cache

    masks = _build_masks()
    hsd = []
    for b in range(B):
        ht = hidden_states[b].T.astype(bf16)                     # [H, S]
        t4 = ht.reshape(16, 128, 4, 512).transpose(2, 1, 0, 3)   # [ch, p, t, n]
        hsd.append(np.ascontiguousarray(t4.reshape(4, 128, 8192)))
    in_maps = []
    for b in range(B):
        for gi in range(KV_HEADS):
            wq = Wq[:, gi * GD:(gi + 1) * GD].astype(bf16)       # [2048, 512]
            wk = Wk[:, gi * D:(gi + 1) * D].astype(bf16)         # [2048, 128]
            wv = Wv[:, gi * D:(gi + 1) * D].astype(bf16)
            wo = Wo[gi * GD:(gi + 1) * GD, :].astype(bf16)       # [512, 2048]
            in_maps.append({
                "hsd": hsd[b],
                "wqd": np.ascontiguousarray(
                    wq.reshape(16, 128, 512).transpose(1, 0, 2).reshape(128, 8192)),
                "wkd": np.ascontiguousarray(
                    wk.reshape(16, 128, 128).transpose(1, 0, 2).reshape(128, 2048)),
                "wvd": np.ascontiguousarray(
                    wv.reshape(16, 128, 128).transpose(1, 0, 2).reshape(128, 2048)),
                "wod": np.ascontiguousarray(
                    wo.reshape(4, 128, 2048).transpose(1, 0, 2).reshape(128, 8192)),
                "mskd": masks,
            })
    res = run_bass_kernel_spmd(nc, in_maps, list(range(8)))
    out = np.zeros((B, S, H), np.float32)
    for b in range(B):
        acc = None
        for gi in range(KV_HEADS):
            o = np.asarray(res.results[b * KV_HEADS + gi]["outd"]).astype(np.float32)
            acc = o if acc is None else acc + o
        out[b] = acc.reshape(S, H)                               # [16,128,2048] -> [S,H]
    return out
